# revision 28
# baseline (speedup 1.0000x reference)
"""Multi-head self-attention kernel for 8 Trainium2 NeuronCores.

Sharding: core c = (b, g) with b = batch index (4), g = head-group (2).
Each core computes attention for one batch element and 8 of the 16 heads,
including its slice of the QKV projections and a partial out-projection.
The host sums the two head-group partials per batch and transposes (the
device produces Y^T).

v2 design (vs the f32r baseline):
- bf16 for x, Wq/Wk/Wv/Wo, Q^T, K^T, O^T: same PE rate (1 cyc/row), half
  the DMA/SBUF traffic, FWL-eligible weight loads.
- fp8e4m3 for P = exp(S) and V, enabling DoubleRow PV matmuls (0.5
  cyc/row, contraction 256 over kc-pairs).  The softmax denominator
  rides along as a ones-column inside the fp8 V blocks (col 64 of each
  80-byte half-block), accumulated in fp32 PSUM, so numerator and
  denominator see the same fp8 rounding.
- One ACT exp instruction per [128 x 1024] spanning two PSUM banks
  (halves the per-instruction overhead of the exp stream, which is the
  binding engine at ~(N+352)/1.2 ns per instruction).
- K/V/Q/out-projection matmul groups are interleaved into the attention
  iteration stream with explicit due-iteration deadlines, keeping the PE
  continuously busy (p-state) without starving the ACT exp stream.
"""

import sys

sys.path.insert(0, "/opt/trn_rl_repo")

from contextlib import ExitStack

import numpy as np
import ml_dtypes

import concourse.bass as bass
import concourse.tile as tile
from concourse import bacc, mybir
from concourse.bass_utils import run_bass_kernel_spmd

F32 = mybir.dt.float32
BF16 = mybir.dt.bfloat16
FP8 = mybir.dt.float8e4
P = 128  # SBUF partitions

D_MODEL = 1024
NHEAD = 16
DK = D_MODEL // NHEAD  # 64
BATCH = 4
SEQ = 2048
N_CORES = 8
HL = NHEAD // 2       # heads per core (head-group of 8)
NPAIR = HL // 2       # head pairs per core (4)
DC = D_MODEL // P     # contraction chunks for projections (8)
KC = SEQ // P         # k chunks of 128 (16)
KCP = KC // 2         # kc-pairs (8)
QB = 512              # q block
NQB = SEQ // QB       # q blocks (4)
HD = HL * DK          # local head-dim total (512)
NOC = D_MODEL // P    # out-dim chunks (8)
VBLK = 160            # per-head fp8 V block: 2 x (64 dims + ones + 15 pad)
EXP_SCALE = 1.0 / np.sqrt(DK)
ITERS_PER_PAIR = 2 * KCP  # 16


def build_bass(repeat=1):
    """Build the per-core Bass program (same program on all 8 cores)."""
    nc = bacc.Bacc("TRN2", target_bir_lowering=False, debug=False,
                   num_devices=N_CORES)

    xT = nc.dram_tensor("xT", [D_MODEL, SEQ], BF16, kind="ExternalInput")
    Wq = nc.dram_tensor("Wq", [D_MODEL, HD], BF16, kind="ExternalInput")
    Wk = nc.dram_tensor("Wk", [D_MODEL, HD], BF16, kind="ExternalInput")
    Wv = nc.dram_tensor("Wv", [D_MODEL, HD], BF16, kind="ExternalInput")
    Wo = nc.dram_tensor("Wo", [HD, D_MODEL], BF16, kind="ExternalInput")
    bq_t = nc.dram_tensor("bq_t", [P, NPAIR], F32, kind="ExternalInput")
    bk_t = nc.dram_tensor("bk_t", [P, NPAIR], F32, kind="ExternalInput")
    bv_bc = nc.dram_tensor("bv_bc", [P, HD], F32, kind="ExternalInput")
    bo_t = nc.dram_tensor("bo_t", [P, NOC], F32, kind="ExternalInput")
    YT = nc.dram_tensor("YT", [D_MODEL, SEQ], F32, kind="ExternalOutput")
    dram = dict(xT=xT, Wq=Wq, Wk=Wk, Wv=Wv, Wo=Wo, bq_t=bq_t, bk_t=bk_t,
                bv_bc=bv_bc, bo_t=bo_t, YT=YT)

    with tile.TileContext(nc) as tc, ExitStack() as ctx:
        consts = ctx.enter_context(tc.tile_pool(name="consts", bufs=1))
        wres = ctx.enter_context(tc.tile_pool(name="wres", bufs=1))
        xres = ctx.enter_context(tc.tile_pool(name="xres", bufs=2))
        ktv = ctx.enter_context(tc.tile_pool(name="ktv", bufs=2))
        qtp = ctx.enter_context(tc.tile_pool(name="qtp", bufs=3))
        pexp = ctx.enter_context(tc.tile_pool(name="pexp", bufs=4))
        otp = ctx.enter_context(tc.tile_pool(name="otp", bufs=8))
        misc = ctx.enter_context(tc.tile_pool(name="misc", bufs=2))
        stage = ctx.enter_context(tc.tile_pool(name="stage", bufs=1))
        ps_s = ctx.enter_context(tc.tile_pool(name="ps_s", bufs=2,
                                              space="PSUM"))
        ps_acc = ctx.enter_context(tc.tile_pool(name="ps_acc", bufs=2,
                                                space="PSUM"))
        ps_p = ctx.enter_context(tc.tile_pool(name="ps_p", bufs=2,
                                              space="PSUM"))

        # ---- constants (tiles now; DMAs ride inside body 0's queue) ----
        bq_sb = consts.tile([P, NPAIR], F32, tag="bq")
        bk_sb = consts.tile([P, NPAIR], F32, tag="bk")
        bv_sb = consts.tile([P, HD], F32, tag="bv")
        bo_sb = consts.tile([P, NOC], F32, tag="bo")

        # warm the ACT exp table early so the first real exp is cheap
        warm = consts.tile([1, 2], F32, tag="warm")
        nc.gpsimd.memset(warm[0:1, 0:1], 0.0)
        nc.scalar.activation(warm[0:1, 1:2], warm[0:1, 0:1],
                             mybir.ActivationFunctionType.Exp)

        # warm the PE p-state: ~24 dependency-free matmuls keep the PE busy
        # through the initial DMA phase so the first projections run at
        # full clock (TRN2 throttles the PE for ~3us after any idle gap)
        wsrc = consts.tile([P, QB], BF16, tag="wsrc")
        nc.vector.memset(wsrc[:], 0.0)

        # ---- weight tiles (allocated now; DMAs emitted inside body 0 so
        # the x window-0 transfer wins the DMA queue) ----
        wk_sb = wres.tile([P, DC * HD], BF16, tag="wk", name="wk")
        wv_sb = wres.tile([P, DC * HD], BF16, tag="wv", name="wv")
        wq_sb = wres.tile([P, DC * HD], BF16, tag="wq", name="wq")
        wo_sb = wres.tile([P, NPAIR * D_MODEL], BF16, tag="wo", name="wo")

        def emit_wdma(which):
            if which == "kqv":
                for t, d in ((wk_sb, Wk), (wv_sb, Wv), (wq_sb, Wq)):
                    nc.sync.dma_start(
                        t[:].rearrange("p (c n) -> p c n", c=DC),
                        d.ap().rearrange("(c p) n -> p c n", p=P))
                nc.sync.dma_start(bk_sb[:], bk_t.ap())
                nc.sync.dma_start(bv_sb[:], bv_bc.ap())
                nc.sync.dma_start(bq_sb[:], bq_t.ap())
                nc.sync.dma_start(bo_sb[:], bo_t.ap())
            elif which == "warmpe":
                for i in range(24):
                    wps = ps_p.tile([P, QB], F32, tag="pp", name="wps")
                    nc.tensor.matmul(wps[:], wsrc[:, 0:P], wsrc[:],
                                     start=True, stop=True)
            else:
                nc.sync.dma_start(
                    wo_sb[:].rearrange("p (r n) -> p r n", r=NPAIR),
                    Wo.ap().rearrange("(r p) n -> p r n", p=P))

        sbs = dict(bq=bq_sb, bk=bk_sb, bv=bv_sb, bo=bo_sb,
                   wq=wq_sb, wk=wk_sb, wv=wv_sb, wo=wo_sb)
        pools = dict(wres=wres, xres=xres, ktv=ktv, qtp=qtp, pexp=pexp,
                     otp=otp, misc=misc, stage=stage, ps_s=ps_s,
                     ps_acc=ps_acc, ps_p=ps_p)

        pctx = None
        for rep in range(repeat):
            pctx = emit_body(nc, tc, dram, sbs, pools, pctx,
                             is_last=(rep == repeat - 1),
                             emit_wdma=emit_wdma if rep == 0 else None)
        for fn in pctx["carry"]:
            fn()

    nc.compile()
    return nc


def emit_body(nc, tc, dram, sbs, pools, pctx, is_last, emit_wdma=None):
    xres, ktv, qtp = pools["xres"], pools["ktv"], pools["qtp"]
    pexp, otp, misc = pools["pexp"], pools["otp"], pools["misc"]
    stage = pools["stage"]
    ps_s, ps_acc, ps_p = pools["ps_s"], pools["ps_acc"], pools["ps_p"]
    bq_sb, bk_sb, bv_sb, bo_sb = sbs["bq"], sbs["bk"], sbs["bv"], sbs["bo"]
    wq_sb, wk_sb, wv_sb, wo_sb = sbs["wq"], sbs["wk"], sbs["wv"], sbs["wo"]

    xt_dram3 = dram["xT"].ap().rearrange("(c p) s -> p c s", p=P)
    yt_dram3 = dram["YT"].ap().rearrange("(n p) s -> p n s", p=P)

    def w3(t, cols=HD):
        return t[:].rearrange("p (c n) -> p c n", c=DC)

    wq3, wk3, wv3 = w3(wq_sb), w3(wk_sb), w3(wv_sb)
    wo3 = wo_sb[:].rearrange("p (r n) -> p r n", r=NPAIR)
    bv3 = bv_sb[:].rearrange("p (h d) -> p h d", h=HL)

    def alloc_xkv():
        xt = xres.tile([P, DC * SEQ], BF16, tag="xt", name="xt")
        return (xt[:].rearrange("p (c s) -> p c s", c=DC),
                [ktv.tile([P, SEQ], BF16, tag=f"kt{r}", name=f"kt{r}")
                 for r in range(NPAIR)],
                [ktv.tile([P, HL * VBLK], FP8, tag=f"v2_{k}", name=f"v2_{k}")
                 for k in range(KCP)])

    first = pctx is None
    if first:
        # body 0: own x/K/V produced in its own qb0; weight loads ride the
        # DMA queue right after x window 0 (wo last: first needed ~iter 70)
        xt3, kt_tiles, v2_tiles = alloc_xkv()
        nc.sync.dma_start(xt3[:, :, bass.ts(0, QB)],
                          xt_dram3[:, :, bass.ts(0, QB)])
        if emit_wdma is not None:
            emit_wdma("kqv")
            emit_wdma("warmpe")
        for w in range(1, NQB):
            sl = bass.ts(w, QB)
            nc.sync.dma_start(xt3[:, :, sl], xt_dram3[:, :, sl])
        if emit_wdma is not None:
            emit_wdma("o")
    else:
        # steady state: x/K^T/V and Q(0,0) were prefetched by the previous
        # body during its ACT-rich later q-blocks
        xt3, kt_tiles, v2_tiles = pctx["xt3"], pctx["kt"], pctx["v2"]

    # ---- projection group emitters (PE fillers) ----
    def k_group(pr, w, x3=None, kts=None):
        x3 = xt3 if x3 is None else x3
        kts = kt_tiles if kts is None else kts
        sl = bass.ts(w, QB)
        kps = ps_p.tile([P, QB], F32, tag="pp", name="kps")
        for c in range(DC):
            nc.tensor.matmul(kps[:], wk3[:, c, bass.ts(pr, P)],
                             x3[:, c, sl],
                             start=(c == 0), stop=(c == DC - 1))
        nc.vector.tensor_scalar_add(kts[pr][:, sl], kps[:],
                                    bk_sb[:, pr:pr + 1])

    qt_map = {}
    if not first:
        qt_map[(0, 0)] = pctx["qt00"]

    def q_group(qb, pr, x3=None, store=None):
        x3 = xt3 if x3 is None else x3
        qps = ps_p.tile([P, QB], F32, tag="pp", name="qps")
        for c in range(DC):
            nc.tensor.matmul(qps[:], wq3[:, c, bass.ts(pr, P)],
                             x3[:, c, bass.ts(qb, QB)],
                             start=(c == 0), stop=(c == DC - 1))
        qt = qtp.tile([P, QB], BF16, tag="qt", name="qt")
        nc.vector.tensor_scalar_add(qt[:], qps[:], bq_sb[:, pr:pr + 1])
        if store is None:
            qt_map[(qb, pr)] = qt
        else:
            store[0] = qt

    # V projection in quarter-groups (4 matmuls each) so the PE load can be
    # spread finely between score/PV iterations
    vps_hold = {}

    def v_quarter(kcp, t, half, x3=None, v2s=None):
        x3 = xt3 if x3 is None else x3
        v2s = v2_tiles if v2s is None else v2s
        k = 2 * kcp + t
        if half == 0:
            vps_hold[(kcp, t)] = ps_p.tile([P, HD], F32, tag="pp", name="vps")
        vps = vps_hold[(kcp, t)]
        for c in range(4 * half, 4 * half + 4):
            nc.tensor.matmul(vps[:], x3[:, c, bass.ts(k, P)],
                             wv3[:, c, :],
                             start=(c == 0), stop=(c == DC - 1))
        if half == 1:
            v2t = v2s[kcp]
            v2h = v2t[:].rearrange("p (h z) -> p h z", h=HL)
            nc.vector.tensor_add(
                v2h[:, :, t * 80: t * 80 + DK],
                vps[:].rearrange("p (h d) -> p h d", h=HL), bv3)
            del vps_hold[(kcp, t)]
            if t == 1:
                # ones columns (softmax denominator rides the PV matmul)
                v2g = v2t[:].rearrange("p (g z) -> p g z", g=2 * HL)
                nc.vector.memset(v2g[:, :, DK:DK + 1], 1.0)

    ot_map = {}
    yhalf = {}

    def o_group(qb, n):
        yps = ps_p.tile([P, QB], F32, tag="pp", name="yps")
        for pr in range(NPAIR):
            nc.tensor.matmul(yps[:], wo3[:, pr, bass.ts(n, P)],
                             ot_map[(qb, pr)][:],
                             start=(pr == 0), stop=(pr == NPAIR - 1))
        ysb = misc.tile([P, QB], F32, tag=f"ysb{n % 4}", name="ysb")
        nc.vector.tensor_scalar_add(ysb[:], yps[:], bo_sb[:, n:n + 1])
        nc.sync.dma_start(yt_dram3[:, n, bass.ts(qb, QB)], ysb[:])

    # last q-block's out-projection is split so pairs {0,1} are folded in
    # while pairs {2,3} still run; only the second half lands in the tail
    def o_half1(qb, n):
        yps = ps_p.tile([P, QB], F32, tag="pp", name="yps")
        for pr in (0, 1):
            nc.tensor.matmul(yps[:], wo3[:, pr, bass.ts(n, P)],
                             ot_map[(qb, pr)][:],
                             start=(pr == 0), stop=(pr == 1))
        ya = stage.tile([P, QB], F32, tag=f"ya{n}", name="ya")
        nc.vector.tensor_scalar_add(ya[:], yps[:], bo_sb[:, n:n + 1])
        yhalf[(qb, n)] = ya

    def o_half2(qb, n):
        yps = ps_p.tile([P, QB], F32, tag="pp", name="yps")
        for pr in (2, 3):
            nc.tensor.matmul(yps[:], wo3[:, pr, bass.ts(n, P)],
                             ot_map[(qb, pr)][:],
                             start=(pr == 2), stop=(pr == 3))
        ysb = misc.tile([P, QB], F32, tag=f"ysb{n % 4}", name="ysb")
        nc.vector.tensor_add(ysb[:], yps[:], yhalf[(qb, n)][:])
        nc.sync.dma_start(yt_dram3[:, n, bass.ts(qb, QB)], ysb[:])

    # ---- filler schedule: (due_global_iter, seq, fn) ----
    fillers = []

    def add(due, fn):
        fillers.append((due, len(fillers), fn))

    # previous body's trailing out-projection halves land first (their ot
    # tiles are ready; the PE has slack while ACT drains its exp backlog)
    if not first:
        for fn in pctx["carry"]:
            add(0, fn)

    if first:
        # body 0 produces its own K/V/Q(0,0) just-in-time during qb0
        for w in range(1, NQB):
            add(3 * w - 1, lambda pr=0, w=w: k_group(pr, w))
        for pr in range(1, NPAIR):
            for w in range(NQB):
                add(pr * ITERS_PER_PAIR + 3 * w - 5,
                    lambda pr=pr, w=w: k_group(pr, w))
        for kcp in range(KCP):
            for t in (0, 1):
                for half in (0, 1):
                    idx = 4 * kcp + 2 * t + half
                    add(max(0, (idx - 2) // 2),
                        lambda kcp=kcp, t=t, half=half: v_quarter(kcp, t, half))
    # Q: due a few iters before pair (qb, pr) starts
    for qb in range(NQB):
        for pr in range(NPAIR):
            if (qb, pr) == (0, 0):
                continue
            add((qb * NPAIR + pr) * ITERS_PER_PAIR - 6,
                lambda qb=qb, pr=pr: q_group(qb, pr))
    # out-projection for qb spread across qb+1; last qb: first half during
    # its own pair 2/3, second half carried into the next body
    for qb in range(NQB - 1):
        for n in range(NOC):
            add((qb + 1) * NPAIR * ITERS_PER_PAIR + 8 * n + 4,
                lambda qb=qb, n=n: o_group(qb, n))
    last_pair2 = ((NQB - 1) * NPAIR + 2) * ITERS_PER_PAIR
    for n in range(NOC):
        add(last_pair2 + 3 * n + 2, lambda n=n: o_half1(NQB - 1, n))

    # ---- prefetch the NEXT body's x/K^T/V/Q(0,0) during qb1-qb3 ----
    pctx_next = {"carry": []}
    if not is_last:
        nxt3, nkt, nv2 = alloc_xkv()
        pctx_next.update(xt3=nxt3, kt=nkt, v2=nv2)

        def next_x_dma(w, x3=nxt3):
            sl = bass.ts(w, QB)
            nc.sync.dma_start(x3[:, :, sl], xt_dram3[:, :, sl])

        for w in range(NQB):
            add(64 + 2 * w, lambda w=w: next_x_dma(w))
        for pr in range(NPAIR):
            for w in range(NQB):
                add(128 + 4 * (4 * pr + w),
                    lambda pr=pr, w=w: k_group(pr, w, x3=nxt3, kts=nkt))
        for kcp in range(KCP):
            for t in (0, 1):
                for half in (0, 1):
                    idx = 4 * kcp + 2 * t + half
                    add(192 + 2 * idx,
                        lambda kcp=kcp, t=t, half=half:
                            v_quarter(kcp, t, half, x3=nxt3, v2s=nv2))
        qt00_box = [None]
        add(250, lambda: q_group(0, 0, x3=nxt3, store=qt00_box))
        pctx_next["qt00_box"] = qt00_box

    fillers.sort()
    fidx = [0]

    def emit_due(it):
        while fidx[0] < len(fillers) and fillers[fidx[0]][0] <= it:
            fillers[fidx[0]][2]()
            fidx[0] += 1

    # ---- normalize: O^T rows 0:64 scaled by 1/denominator (row 64) ----
    def normalize(qb, pr, hh, acc):
        if hh == 0:
            ot_map[(qb, pr)] = otp.tile([P, QB], BF16, tag="ot", name="ot")
        ra = misc.tile([1, QB], F32, tag="ra", name="ra")
        nc.vector.reciprocal(ra[:], acc[DK:DK + 1, :])
        bc = misc.tile([DK, QB], F32, tag="bc", name="bc")
        nc.gpsimd.partition_broadcast(bc[:], ra[:], channels=DK)
        nc.vector.tensor_mul(ot_map[(qb, pr)][hh * DK:(hh + 1) * DK, :],
                             acc[0:DK, :], bc[:])

    # ---- main attention stream ----
    state = {"pending": None}
    acc_by_hh = {}

    def flush_pending():
        p = state["pending"]
        if p is None:
            return
        ef, qb, pr, hh, kcp = p
        if kcp == 0:
            acc_by_hh[hh] = ps_acc.tile([DK + 1, QB], F32, tag="acc",
                                        name="acc")
        acc = acc_by_hh[hh]
        h = 2 * pr + hh
        v2t = v2_tiles[kcp]
        lhsT = (v2t[:, h * VBLK:(h + 1) * VBLK]
                .rearrange("p (t c) -> p t c", t=2)[:, :, 0:DK + 1])
        rhs = ef[:].rearrange("p (t n) -> p t n", t=2)
        nc.tensor.matmul(acc[:], lhsT, rhs, start=(kcp == 0),
                         stop=(kcp == KCP - 1),
                         perf_mode=mybir.MatmulPerfMode.DoubleRow)
        if kcp == KCP - 1:
            normalize(qb, pr, hh, acc)
        state["pending"] = None

    # prologue: body 0 needs its first K window + first Q before any scores
    # (steady-state bodies got them via prefetch)
    if first:
        k_group(0, 0)
        q_group(0, 0)

    git = 0
    for qb in range(NQB):
        for pr in range(NPAIR):
            for it_p in range(ITERS_PER_PAIR):
                kcp, hh = it_p // 2, it_p % 2
                flush_pending()
                qt = qt_map[(qb, pr)]
                swide = ps_s.tile([P, 2 * QB], F32, tag="sw", name="sw")
                hsl = slice(hh * DK, (hh + 1) * DK)
                for t in (0, 1):
                    k = 2 * kcp + t
                    nc.tensor.matmul(swide[:, t * QB:(t + 1) * QB],
                                     kt_tiles[pr][hsl, bass.ts(k, P)],
                                     qt[hsl, :], start=True, stop=True)
                ef = pexp.tile([P, 2 * QB], FP8, tag="ef", name="ef")
                nc.scalar.activation(ef[:], swide[:],
                                     mybir.ActivationFunctionType.Exp,
                                     scale=float(EXP_SCALE))
                state["pending"] = (ef, qb, pr, hh, kcp)
                emit_due(git)
                git += 1

    # epilogue: last PV + normalize, remaining fillers; the trailing
    # out-projection halves carry into the next body's filler stream
    flush_pending()
    emit_due(1 << 30)
    if not is_last:
        pctx_next["qt00"] = pctx_next.pop("qt00_box")[0]
    pctx_next["carry"] = [
        (lambda n=n: o_half2(NQB - 1, n)) for n in range(NOC)]
    return pctx_next


_CACHE = {}


def _get_nc():
    if "nc" not in _CACHE:
        _CACHE["nc"] = build_bass()
    return _CACHE["nc"]


def host_prep(x, Wq, bq, Wk, bk, Wv, bv, Wo, bo):
    """Build the 8 per-core input maps."""
    bf = ml_dtypes.bfloat16
    in_maps = []
    for core in range(N_CORES):
        b, g = divmod(core, 2)
        lo, hi = g * HD, (g + 1) * HD
        in_maps.append({
            "xT": np.ascontiguousarray(x[b].T).astype(bf),
            "Wq": np.ascontiguousarray(Wq[:, lo:hi]).astype(bf),
            "Wk": np.ascontiguousarray(Wk[:, lo:hi]).astype(bf),
            "Wv": np.ascontiguousarray(Wv[:, lo:hi]).astype(bf),
            "Wo": np.ascontiguousarray(Wo[lo:hi, :]).astype(bf),
            "bq_t": np.ascontiguousarray(bq[lo:hi].reshape(NPAIR, P).T),
            "bk_t": np.ascontiguousarray(bk[lo:hi].reshape(NPAIR, P).T),
            "bv_bc": np.broadcast_to(bv[lo:hi], (P, HD)).copy(),
            "bo_t": np.ascontiguousarray((bo * 0.5).reshape(NOC, P).T),
        })
    return in_maps


def host_gather(results):
    """Sum head-group partials and transpose back to [B, S, D]."""
    out = np.empty((BATCH, SEQ, D_MODEL), dtype=np.float32)
    for b in range(BATCH):
        yt = results[2 * b]["YT"] + results[2 * b + 1]["YT"]
        out[b] = yt.T
    return out


def kernel(x, Wq, bq, Wk, bk, Wv, bv, Wo, bo):
    nc = _get_nc()
    in_maps = host_prep(x, Wq, bq, Wk, bk, Wv, bv, Wo, bo)
    res = run_bass_kernel_spmd(nc, in_maps, core_ids=list(range(N_CORES)))
    return host_gather(res.results)


# revision 30
# speedup vs baseline: 1.8159x; 1.8159x over previous
"""Multi-head self-attention kernel for 8 Trainium2 NeuronCores.

Sharding: core c = (b, g) with b = batch index (4), g = head-group (2).
Each core computes attention for one batch element and 8 of the 16 heads,
including its slice of the QKV projections and a partial out-projection.
The host sums the two head-group partials per batch and transposes (the
device produces Y^T).

v2 design (vs the f32r baseline):
- bf16 for x, Wq/Wk/Wv/Wo, Q^T, K^T, O^T: same PE rate (1 cyc/row), half
  the DMA/SBUF traffic, FWL-eligible weight loads.
- fp8e4m3 for P = exp(S) and V, enabling DoubleRow PV matmuls (0.5
  cyc/row, contraction 256 over kc-pairs).  The softmax denominator
  rides along as a ones-column inside the fp8 V blocks (col 64 of each
  80-byte half-block), accumulated in fp32 PSUM, so numerator and
  denominator see the same fp8 rounding.
- One ACT exp instruction per [128 x 1024] spanning two PSUM banks
  (halves the per-instruction overhead of the exp stream, which is the
  binding engine at ~(N+352)/1.2 ns per instruction).
- K/V/Q/out-projection matmul groups are interleaved into the attention
  iteration stream with explicit due-iteration deadlines, keeping the PE
  continuously busy (p-state) without starving the ACT exp stream.
"""

import sys

sys.path.insert(0, "/opt/trn_rl_repo")

from contextlib import ExitStack

import numpy as np
import ml_dtypes

import concourse.bass as bass
import concourse.tile as tile
from concourse import bacc, mybir
from concourse.bass_utils import run_bass_kernel_spmd

F32 = mybir.dt.float32
BF16 = mybir.dt.bfloat16
FP8 = mybir.dt.float8e4
P = 128  # SBUF partitions

D_MODEL = 1024
NHEAD = 16
DK = D_MODEL // NHEAD  # 64
BATCH = 4
SEQ = 2048
N_CORES = 8
HL = NHEAD // 2       # heads per core (head-group of 8)
NPAIR = HL // 2       # head pairs per core (4)
DC = D_MODEL // P     # contraction chunks for projections (8)
KC = SEQ // P         # k chunks of 128 (16)
KCP = KC // 2         # kc-pairs (8)
QB = 512              # q block
NQB = SEQ // QB       # q blocks (4)
HD = HL * DK          # local head-dim total (512)
NOC = D_MODEL // P    # out-dim chunks (8)
VBLK = 160            # per-head fp8 V block: 2 x (64 dims + ones + 15 pad)
EXP_SCALE = 1.0 / np.sqrt(DK)
ITERS_PER_PAIR = 2 * KCP  # 16


def build_bass(repeat=1):
    """Build the per-core Bass program (same program on all 8 cores)."""
    nc = bacc.Bacc("TRN2", target_bir_lowering=False, debug=False,
                   num_devices=N_CORES)

    xT = nc.dram_tensor("xT", [D_MODEL, SEQ], BF16, kind="ExternalInput")
    Wq = nc.dram_tensor("Wq", [D_MODEL, HD], BF16, kind="ExternalInput")
    Wk = nc.dram_tensor("Wk", [D_MODEL, HD], BF16, kind="ExternalInput")
    Wv = nc.dram_tensor("Wv", [D_MODEL, HD], BF16, kind="ExternalInput")
    Wo = nc.dram_tensor("Wo", [HD, D_MODEL], BF16, kind="ExternalInput")
    bq_t = nc.dram_tensor("bq_t", [P, NPAIR], F32, kind="ExternalInput")
    bk_t = nc.dram_tensor("bk_t", [P, NPAIR], F32, kind="ExternalInput")
    bv_bc = nc.dram_tensor("bv_bc", [P, HD], F32, kind="ExternalInput")
    bo_t = nc.dram_tensor("bo_t", [P, NOC], F32, kind="ExternalInput")
    YT = nc.dram_tensor("YT", [D_MODEL, SEQ], F32, kind="ExternalOutput")
    dram = dict(xT=xT, Wq=Wq, Wk=Wk, Wv=Wv, Wo=Wo, bq_t=bq_t, bk_t=bk_t,
                bv_bc=bv_bc, bo_t=bo_t, YT=YT)

    with tile.TileContext(nc) as tc, ExitStack() as ctx:
        consts = ctx.enter_context(tc.tile_pool(name="consts", bufs=1))
        wres = ctx.enter_context(tc.tile_pool(name="wres", bufs=1))
        xres = ctx.enter_context(tc.tile_pool(name="xres", bufs=2))
        ktv = ctx.enter_context(tc.tile_pool(name="ktv", bufs=2))
        qtp = ctx.enter_context(tc.tile_pool(name="qtp", bufs=3))
        pexp = ctx.enter_context(tc.tile_pool(name="pexp", bufs=4))
        otp = ctx.enter_context(tc.tile_pool(name="otp", bufs=8))
        misc = ctx.enter_context(tc.tile_pool(name="misc", bufs=2))
        stage = ctx.enter_context(tc.tile_pool(name="stage", bufs=1))
        ps_s = ctx.enter_context(tc.tile_pool(name="ps_s", bufs=2,
                                              space="PSUM"))
        ps_acc = ctx.enter_context(tc.tile_pool(name="ps_acc", bufs=2,
                                                space="PSUM"))
        ps_p = ctx.enter_context(tc.tile_pool(name="ps_p", bufs=2,
                                              space="PSUM"))

        # ---- constants (tiles now; DMAs ride inside body 0's queue) ----
        bq_sb = consts.tile([P, NPAIR], F32, tag="bq")
        bk_sb = consts.tile([P, NPAIR], F32, tag="bk")
        bv_sb = consts.tile([P, HD], F32, tag="bv")
        bo_sb = consts.tile([P, NOC], F32, tag="bo")

        # warm the ACT exp table early so the first real exp is cheap
        warm = consts.tile([1, 2], F32, tag="warm")
        nc.gpsimd.memset(warm[0:1, 0:1], 0.0)
        nc.scalar.activation(warm[0:1, 1:2], warm[0:1, 0:1],
                             mybir.ActivationFunctionType.Exp)

        # warm the PE p-state: ~24 dependency-free matmuls keep the PE busy
        # through the initial DMA phase so the first projections run at
        # full clock (TRN2 throttles the PE for ~3us after any idle gap)
        wsrc = consts.tile([P, QB], BF16, tag="wsrc")
        nc.vector.memset(wsrc[:], 0.0)

        # ---- weight tiles (allocated now; DMAs emitted inside body 0 so
        # the x window-0 transfer wins the DMA queue) ----
        wk_sb = wres.tile([P, DC * HD], BF16, tag="wk", name="wk")
        wv_sb = wres.tile([P, DC * HD], BF16, tag="wv", name="wv")
        wq_sb = wres.tile([P, DC * HD], BF16, tag="wq", name="wq")
        wo_sb = wres.tile([P, NPAIR * D_MODEL], BF16, tag="wo", name="wo")

        def emit_wdma(which):
            if which == "kqv":
                for t, d in ((wk_sb, Wk), (wv_sb, Wv), (wq_sb, Wq)):
                    nc.sync.dma_start(
                        t[:].rearrange("p (c n) -> p c n", c=DC),
                        d.ap().rearrange("(c p) n -> p c n", p=P))
                nc.sync.dma_start(bk_sb[:], bk_t.ap())
                nc.sync.dma_start(bv_sb[:], bv_bc.ap())
                nc.sync.dma_start(bq_sb[:], bq_t.ap())
                nc.sync.dma_start(bo_sb[:], bo_t.ap())
            elif which == "warmpe":
                for i in range(24):
                    wps = ps_p.tile([P, QB], F32, tag="pp", name="wps")
                    nc.tensor.matmul(wps[:], wsrc[:, 0:P], wsrc[:],
                                     start=True, stop=True)
            else:
                nc.sync.dma_start(
                    wo_sb[:].rearrange("p (r n) -> p r n", r=NPAIR),
                    Wo.ap().rearrange("(r p) n -> p r n", p=P))

        sbs = dict(bq=bq_sb, bk=bk_sb, bv=bv_sb, bo=bo_sb,
                   wq=wq_sb, wk=wk_sb, wv=wv_sb, wo=wo_sb)
        pools = dict(wres=wres, xres=xres, ktv=ktv, qtp=qtp, pexp=pexp,
                     otp=otp, misc=misc, stage=stage, ps_s=ps_s,
                     ps_acc=ps_acc, ps_p=ps_p)

        pctx = None
        for rep in range(repeat):
            pctx = emit_body(nc, tc, dram, sbs, pools, pctx,
                             is_last=(rep == repeat - 1),
                             emit_wdma=emit_wdma if rep == 0 else None)
        for fn in pctx["carry"]:
            fn()

    nc.compile()
    return nc


def emit_body(nc, tc, dram, sbs, pools, pctx, is_last, emit_wdma=None):
    xres, ktv, qtp = pools["xres"], pools["ktv"], pools["qtp"]
    pexp, otp, misc = pools["pexp"], pools["otp"], pools["misc"]
    stage = pools["stage"]
    ps_s, ps_acc, ps_p = pools["ps_s"], pools["ps_acc"], pools["ps_p"]
    bq_sb, bk_sb, bv_sb, bo_sb = sbs["bq"], sbs["bk"], sbs["bv"], sbs["bo"]
    wq_sb, wk_sb, wv_sb, wo_sb = sbs["wq"], sbs["wk"], sbs["wv"], sbs["wo"]

    xt_dram3 = dram["xT"].ap().rearrange("(c p) s -> p c s", p=P)
    yt_dram3 = dram["YT"].ap().rearrange("(n p) s -> p n s", p=P)

    def w3(t, cols=HD):
        return t[:].rearrange("p (c n) -> p c n", c=DC)

    wq3, wk3, wv3 = w3(wq_sb), w3(wk_sb), w3(wv_sb)
    wo3 = wo_sb[:].rearrange("p (r n) -> p r n", r=NPAIR)
    bv3 = bv_sb[:].rearrange("p (h d) -> p h d", h=HL)

    def alloc_xkv():
        xt = xres.tile([P, DC * SEQ], BF16, tag="xt", name="xt")
        return (xt[:].rearrange("p (c s) -> p c s", c=DC),
                [ktv.tile([P, SEQ], BF16, tag=f"kt{r}", name=f"kt{r}")
                 for r in range(NPAIR)],
                [ktv.tile([P, HL * VBLK], FP8, tag=f"v2_{k}", name=f"v2_{k}")
                 for k in range(KCP)])

    first = pctx is None
    if first:
        # body 0: own x/K/V produced in its own qb0; weight loads ride the
        # DMA queue right after x window 0 (wo last: first needed ~iter 70)
        xt3, kt_tiles, v2_tiles = alloc_xkv()
        nc.sync.dma_start(xt3[:, :, bass.ts(0, QB)],
                          xt_dram3[:, :, bass.ts(0, QB)])
        if emit_wdma is not None:
            emit_wdma("kqv")
            emit_wdma("warmpe")
        for w in range(1, NQB):
            sl = bass.ts(w, QB)
            nc.sync.dma_start(xt3[:, :, sl], xt_dram3[:, :, sl])
        if emit_wdma is not None:
            emit_wdma("o")
    else:
        # steady state: x/K^T/V and Q(0,0) were prefetched by the previous
        # body during its ACT-rich later q-blocks
        xt3, kt_tiles, v2_tiles = pctx["xt3"], pctx["kt"], pctx["v2"]

    # ---- projection group emitters (PE fillers) ----
    # All groups come in 4-matmul halves so no single filler exceeds ~0.9us
    # of PE time — the exp stream's PSUM runway is only ~2 tiles deep.
    kps_hold = {}

    def k_half(pr, w, half, x3=None, kts=None):
        x3 = xt3 if x3 is None else x3
        kts = kt_tiles if kts is None else kts
        sl = bass.ts(w, QB)
        if half == 0:
            kps_hold[(pr, w)] = ps_p.tile([P, QB], F32, tag="pp", name="kps")
        kps = kps_hold[(pr, w)]
        for c in range(4 * half, 4 * half + 4):
            nc.tensor.matmul(kps[:], wk3[:, c, bass.ts(pr, P)],
                             x3[:, c, sl],
                             start=(c == 0), stop=(c == DC - 1))
        if half == 1:
            nc.vector.tensor_scalar_add(kts[pr][:, sl], kps[:],
                                        bk_sb[:, pr:pr + 1])
            del kps_hold[(pr, w)]

    def k_group(pr, w, x3=None, kts=None):
        k_half(pr, w, 0, x3, kts)
        k_half(pr, w, 1, x3, kts)

    qt_map = {}
    if not first:
        qt_map[(0, 0)] = pctx["qt00"]
    qps_hold = {}

    def q_half(qb, pr, half, x3=None, store=None):
        x3 = xt3 if x3 is None else x3
        if half == 0:
            qps_hold[(qb, pr)] = ps_p.tile([P, QB], F32, tag="pp",
                                           name="qps")
        qps = qps_hold[(qb, pr)]
        for c in range(4 * half, 4 * half + 4):
            nc.tensor.matmul(qps[:], wq3[:, c, bass.ts(pr, P)],
                             x3[:, c, bass.ts(qb, QB)],
                             start=(c == 0), stop=(c == DC - 1))
        if half == 1:
            qt = qtp.tile([P, QB], BF16, tag="qt", name="qt")
            nc.vector.tensor_scalar_add(qt[:], qps[:], bq_sb[:, pr:pr + 1])
            del qps_hold[(qb, pr)]
            if store is None:
                qt_map[(qb, pr)] = qt
            else:
                store[0] = qt

    def q_group(qb, pr, x3=None, store=None):
        q_half(qb, pr, 0, x3, store)
        q_half(qb, pr, 1, x3, store)

    # V projection in quarter-groups (4 matmuls each) so the PE load can be
    # spread finely between score/PV iterations
    vps_hold = {}

    def v_quarter(kcp, t, half, x3=None, v2s=None):
        x3 = xt3 if x3 is None else x3
        v2s = v2_tiles if v2s is None else v2s
        k = 2 * kcp + t
        if half == 0:
            vps_hold[(kcp, t)] = ps_p.tile([P, HD], F32, tag="pp", name="vps")
        vps = vps_hold[(kcp, t)]
        for c in range(4 * half, 4 * half + 4):
            nc.tensor.matmul(vps[:], x3[:, c, bass.ts(k, P)],
                             wv3[:, c, :],
                             start=(c == 0), stop=(c == DC - 1))
        if half == 1:
            v2t = v2s[kcp]
            v2h = v2t[:].rearrange("p (h z) -> p h z", h=HL)
            nc.vector.tensor_add(
                v2h[:, :, t * 80: t * 80 + DK],
                vps[:].rearrange("p (h d) -> p h d", h=HL), bv3)
            del vps_hold[(kcp, t)]
            if t == 1:
                # ones columns (softmax denominator rides the PV matmul)
                v2g = v2t[:].rearrange("p (g z) -> p g z", g=2 * HL)
                nc.vector.memset(v2g[:, :, DK:DK + 1], 1.0)

    ot_map = {}
    yhalf = {}

    def o_group(qb, n):
        yps = ps_p.tile([P, QB], F32, tag="pp", name="yps")
        for pr in range(NPAIR):
            nc.tensor.matmul(yps[:], wo3[:, pr, bass.ts(n, P)],
                             ot_map[(qb, pr)][:],
                             start=(pr == 0), stop=(pr == NPAIR - 1))
        ysb = misc.tile([P, QB], F32, tag=f"ysb{n % 4}", name="ysb")
        nc.vector.tensor_scalar_add(ysb[:], yps[:], bo_sb[:, n:n + 1])
        nc.sync.dma_start(yt_dram3[:, n, bass.ts(qb, QB)], ysb[:])

    # last q-block's out-projection is split so pairs {0,1} are folded in
    # while pairs {2,3} still run; only the second half lands in the tail
    def o_half1(qb, n):
        yps = ps_p.tile([P, QB], F32, tag="pp", name="yps")
        for pr in (0, 1):
            nc.tensor.matmul(yps[:], wo3[:, pr, bass.ts(n, P)],
                             ot_map[(qb, pr)][:],
                             start=(pr == 0), stop=(pr == 1))
        ya = stage.tile([P, QB], F32, tag=f"ya{n}", name="ya")
        nc.vector.tensor_scalar_add(ya[:], yps[:], bo_sb[:, n:n + 1])
        yhalf[(qb, n)] = ya

    def o_half2(qb, n):
        yps = ps_p.tile([P, QB], F32, tag="pp", name="yps")
        for pr in (2, 3):
            nc.tensor.matmul(yps[:], wo3[:, pr, bass.ts(n, P)],
                             ot_map[(qb, pr)][:],
                             start=(pr == 2), stop=(pr == 3))
        ysb = misc.tile([P, QB], F32, tag=f"ysb{n % 4}", name="ysb")
        nc.vector.tensor_add(ysb[:], yps[:], yhalf[(qb, n)][:])
        nc.sync.dma_start(yt_dram3[:, n, bass.ts(qb, QB)], ysb[:])

    # ---- filler schedule: (due_global_iter, seq, fn) ----
    fillers = []

    def add(due, fn):
        fillers.append((due, len(fillers), fn))

    # previous body's trailing out-projection halves land first (their ot
    # tiles are ready; the PE has slack while ACT drains its exp backlog)
    if not first:
        for fn in pctx["carry"]:
            add(0, fn)

    if first:
        # body 0 produces its own K/V/Q(0,0) just-in-time during qb0
        for w in range(1, NQB):
            for half in (0, 1):
                add(3 * w - 1 + half, lambda pr=0, w=w, half=half:
                    k_half(pr, w, half))
        for pr in range(1, NPAIR):
            for w in range(NQB):
                for half in (0, 1):
                    add(pr * ITERS_PER_PAIR + 3 * w - 5 + half,
                        lambda pr=pr, w=w, half=half: k_half(pr, w, half))
        for kcp in range(KCP):
            for t in (0, 1):
                for half in (0, 1):
                    idx = 4 * kcp + 2 * t + half
                    add(max(0, (idx - 2) // 2),
                        lambda kcp=kcp, t=t, half=half: v_quarter(kcp, t, half))
    # Q: due a few iters before pair (qb, pr) starts
    for qb in range(NQB):
        for pr in range(NPAIR):
            if (qb, pr) == (0, 0):
                continue
            base = (qb * NPAIR + pr) * ITERS_PER_PAIR - 7
            add(base, lambda qb=qb, pr=pr: q_half(qb, pr, 0))
            add(base + 1, lambda qb=qb, pr=pr: q_half(qb, pr, 1))
    # out-projection for qb spread across qb+1 (odd offsets so they don't
    # land on the same iteration as the K/V prefetch fillers); last qb:
    # first half during its own pair 2/3, second half carried into the
    # next body
    for qb in range(NQB - 1):
        for n in range(NOC):
            add((qb + 1) * NPAIR * ITERS_PER_PAIR + 8 * n + 5,
                lambda qb=qb, n=n: o_group(qb, n))
    last_pair2 = ((NQB - 1) * NPAIR + 2) * ITERS_PER_PAIR
    for n in range(NOC):
        add(last_pair2 + 3 * n + 2, lambda n=n: o_half1(NQB - 1, n))

    # ---- prefetch the NEXT body's x/K^T/V/Q(0,0) during qb1-qb3 ----
    pctx_next = {"carry": []}
    if not is_last:
        nxt3, nkt, nv2 = alloc_xkv()
        pctx_next.update(xt3=nxt3, kt=nkt, v2=nv2)

        def next_x_dma(w, x3=nxt3):
            sl = bass.ts(w, QB)
            nc.sync.dma_start(x3[:, :, sl], xt_dram3[:, :, sl])

        for w in range(NQB):
            add(64 + 2 * w, lambda w=w: next_x_dma(w))
        for pr in range(NPAIR):
            for w in range(NQB):
                for half in (0, 1):
                    add(128 + 4 * (4 * pr + w) + 2 * half,
                        lambda pr=pr, w=w, half=half:
                            k_half(pr, w, half, x3=nxt3, kts=nkt))
        for kcp in range(KCP):
            for t in (0, 1):
                for half in (0, 1):
                    idx = 4 * kcp + 2 * t + half
                    add(192 + 2 * idx,
                        lambda kcp=kcp, t=t, half=half:
                            v_quarter(kcp, t, half, x3=nxt3, v2s=nv2))
        qt00_box = [None]
        add(248, lambda: q_half(0, 0, 0, x3=nxt3, store=qt00_box))
        add(249, lambda: q_half(0, 0, 1, x3=nxt3, store=qt00_box))
        pctx_next["qt00_box"] = qt00_box

    fillers.sort()
    fidx = [0]

    def emit_due(it):
        while fidx[0] < len(fillers) and fillers[fidx[0]][0] <= it:
            fillers[fidx[0]][2]()
            fidx[0] += 1

    # ---- normalize: O^T rows 0:64 scaled by 1/denominator (row 64) ----
    def normalize(qb, pr, hh, acc):
        if hh == 0:
            ot_map[(qb, pr)] = otp.tile([P, QB], BF16, tag="ot", name="ot")
        ra = misc.tile([1, QB], F32, tag="ra", name="ra")
        nc.vector.reciprocal(ra[:], acc[DK:DK + 1, :])
        bc = misc.tile([DK, QB], F32, tag="bc", name="bc")
        nc.gpsimd.partition_broadcast(bc[:], ra[:], channels=DK)
        nc.vector.tensor_mul(ot_map[(qb, pr)][hh * DK:(hh + 1) * DK, :],
                             acc[0:DK, :], bc[:])

    # ---- main attention stream ----
    state = {"pending": None}
    acc_by_hh = {}

    def flush_pending():
        p = state["pending"]
        if p is None:
            return
        ef, qb, pr, hh, kcp = p
        if kcp == 0:
            acc_by_hh[hh] = ps_acc.tile([DK + 1, QB], F32, tag="acc",
                                        name="acc")
        acc = acc_by_hh[hh]
        h = 2 * pr + hh
        v2t = v2_tiles[kcp]
        lhsT = (v2t[:, h * VBLK:(h + 1) * VBLK]
                .rearrange("p (t c) -> p t c", t=2)[:, :, 0:DK + 1])
        rhs = ef[:].rearrange("p (t n) -> p t n", t=2)
        nc.tensor.matmul(acc[:], lhsT, rhs, start=(kcp == 0),
                         stop=(kcp == KCP - 1),
                         perf_mode=mybir.MatmulPerfMode.DoubleRow)
        if kcp == KCP - 1:
            normalize(qb, pr, hh, acc)
        state["pending"] = None

    # prologue: body 0 needs its first K window + first Q before any scores
    # (steady-state bodies got them via prefetch)
    if first:
        k_group(0, 0)
        q_group(0, 0)

    git = 0
    for qb in range(NQB):
        for pr in range(NPAIR):
            for it_p in range(ITERS_PER_PAIR):
                kcp, hh = it_p // 2, it_p % 2
                flush_pending()
                qt = qt_map[(qb, pr)]
                swide = ps_s.tile([P, 2 * QB], F32, tag="sw", name="sw")
                hsl = slice(hh * DK, (hh + 1) * DK)
                for t in (0, 1):
                    k = 2 * kcp + t
                    nc.tensor.matmul(swide[:, t * QB:(t + 1) * QB],
                                     kt_tiles[pr][hsl, bass.ts(k, P)],
                                     qt[hsl, :], start=True, stop=True)
                ef = pexp.tile([P, 2 * QB], FP8, tag="ef", name="ef")
                nc.scalar.activation(ef[:], swide[:],
                                     mybir.ActivationFunctionType.Exp,
                                     scale=float(EXP_SCALE))
                state["pending"] = (ef, qb, pr, hh, kcp)
                emit_due(git)
                git += 1

    # epilogue: last PV + normalize, remaining fillers; the trailing
    # out-projection halves carry into the next body's filler stream
    flush_pending()
    emit_due(1 << 30)
    if not is_last:
        pctx_next["qt00"] = pctx_next.pop("qt00_box")[0]
    pctx_next["carry"] = [
        (lambda n=n: o_half2(NQB - 1, n)) for n in range(NOC)]
    return pctx_next


_CACHE = {}


def _get_nc():
    if "nc" not in _CACHE:
        _CACHE["nc"] = build_bass()
    return _CACHE["nc"]


def host_prep(x, Wq, bq, Wk, bk, Wv, bv, Wo, bo):
    """Build the 8 per-core input maps."""
    bf = ml_dtypes.bfloat16
    in_maps = []
    for core in range(N_CORES):
        b, g = divmod(core, 2)
        lo, hi = g * HD, (g + 1) * HD
        in_maps.append({
            "xT": np.ascontiguousarray(x[b].T).astype(bf),
            "Wq": np.ascontiguousarray(Wq[:, lo:hi]).astype(bf),
            "Wk": np.ascontiguousarray(Wk[:, lo:hi]).astype(bf),
            "Wv": np.ascontiguousarray(Wv[:, lo:hi]).astype(bf),
            "Wo": np.ascontiguousarray(Wo[lo:hi, :]).astype(bf),
            "bq_t": np.ascontiguousarray(bq[lo:hi].reshape(NPAIR, P).T),
            "bk_t": np.ascontiguousarray(bk[lo:hi].reshape(NPAIR, P).T),
            "bv_bc": np.broadcast_to(bv[lo:hi], (P, HD)).copy(),
            "bo_t": np.ascontiguousarray((bo * 0.5).reshape(NOC, P).T),
        })
    return in_maps


def host_gather(results):
    """Sum head-group partials and transpose back to [B, S, D]."""
    out = np.empty((BATCH, SEQ, D_MODEL), dtype=np.float32)
    for b in range(BATCH):
        yt = results[2 * b]["YT"] + results[2 * b + 1]["YT"]
        out[b] = yt.T
    return out


def kernel(x, Wq, bq, Wk, bk, Wv, bv, Wo, bo):
    nc = _get_nc()
    in_maps = host_prep(x, Wq, bq, Wk, bk, Wv, bv, Wo, bo)
    res = run_bass_kernel_spmd(nc, in_maps, core_ids=list(range(N_CORES)))
    return host_gather(res.results)


# revision 31
# speedup vs baseline: 2.1089x; 1.1614x over previous
"""Multi-head self-attention kernel for 8 Trainium2 NeuronCores.

Sharding: core c = (b, g) with b = batch index (4), g = head-group (2).
Each core computes attention for one batch element and 8 of the 16 heads,
including its slice of the QKV projections and a partial out-projection.
The host sums the two head-group partials per batch and transposes (the
device produces Y^T).

v2 design (vs the f32r baseline):
- bf16 for x, Wq/Wk/Wv/Wo, Q^T, K^T, O^T: same PE rate (1 cyc/row), half
  the DMA/SBUF traffic, FWL-eligible weight loads.
- fp8e4m3 for P = exp(S) and V, enabling DoubleRow PV matmuls (0.5
  cyc/row, contraction 256 over kc-pairs).  The softmax denominator
  rides along as a ones-column inside the fp8 V blocks (col 64 of each
  80-byte half-block), accumulated in fp32 PSUM, so numerator and
  denominator see the same fp8 rounding.
- One ACT exp instruction per [128 x 1024] spanning two PSUM banks
  (halves the per-instruction overhead of the exp stream, which is the
  binding engine at ~(N+352)/1.2 ns per instruction).
- K/V/Q/out-projection matmul groups are interleaved into the attention
  iteration stream with explicit due-iteration deadlines, keeping the PE
  continuously busy (p-state) without starving the ACT exp stream.
"""

import sys

sys.path.insert(0, "/opt/trn_rl_repo")

from contextlib import ExitStack

import numpy as np
import ml_dtypes

import concourse.bass as bass
import concourse.tile as tile
from concourse import bacc, mybir
from concourse.bass_utils import run_bass_kernel_spmd

F32 = mybir.dt.float32
BF16 = mybir.dt.bfloat16
FP8 = mybir.dt.float8e4
P = 128  # SBUF partitions

D_MODEL = 1024
NHEAD = 16
DK = D_MODEL // NHEAD  # 64
BATCH = 4
SEQ = 2048
N_CORES = 8
HL = NHEAD // 2       # heads per core (head-group of 8)
NPAIR = HL // 2       # head pairs per core (4)
DC = D_MODEL // P     # contraction chunks for projections (8)
KC = SEQ // P         # k chunks of 128 (16)
KCP = KC // 2         # kc-pairs (8)
QB = 512              # q block
NQB = SEQ // QB       # q blocks (4)
HD = HL * DK          # local head-dim total (512)
NOC = D_MODEL // P    # out-dim chunks (8)
VBLK = 160            # per-head fp8 V block: 2 x (64 dims + ones + 15 pad)
EXP_SCALE = 1.0 / np.sqrt(DK)
ITERS_PER_PAIR = 2 * KCP  # 16


def build_bass(repeat=1):
    """Build the per-core Bass program (same program on all 8 cores)."""
    nc = bacc.Bacc("TRN2", target_bir_lowering=False, debug=False,
                   num_devices=N_CORES)

    xT = nc.dram_tensor("xT", [D_MODEL, SEQ], BF16, kind="ExternalInput")
    Wq = nc.dram_tensor("Wq", [D_MODEL, HD], BF16, kind="ExternalInput")
    Wk = nc.dram_tensor("Wk", [D_MODEL, HD], BF16, kind="ExternalInput")
    Wv = nc.dram_tensor("Wv", [D_MODEL, HD], BF16, kind="ExternalInput")
    Wo = nc.dram_tensor("Wo", [HD, D_MODEL], BF16, kind="ExternalInput")
    bq_t = nc.dram_tensor("bq_t", [P, NPAIR], F32, kind="ExternalInput")
    bk_t = nc.dram_tensor("bk_t", [P, NPAIR], F32, kind="ExternalInput")
    bv_bc = nc.dram_tensor("bv_bc", [P, HD], F32, kind="ExternalInput")
    bo_t = nc.dram_tensor("bo_t", [P, NOC], F32, kind="ExternalInput")
    YT = nc.dram_tensor("YT", [D_MODEL, SEQ], F32, kind="ExternalOutput")
    dram = dict(xT=xT, Wq=Wq, Wk=Wk, Wv=Wv, Wo=Wo, bq_t=bq_t, bk_t=bk_t,
                bv_bc=bv_bc, bo_t=bo_t, YT=YT)

    with tile.TileContext(nc) as tc, ExitStack() as ctx:
        consts = ctx.enter_context(tc.tile_pool(name="consts", bufs=1))
        wres = ctx.enter_context(tc.tile_pool(name="wres", bufs=1))
        xres = ctx.enter_context(tc.tile_pool(name="xres", bufs=2))
        ktv = ctx.enter_context(tc.tile_pool(name="ktv", bufs=2))
        qtp = ctx.enter_context(tc.tile_pool(name="qtp", bufs=3))
        pexp = ctx.enter_context(tc.tile_pool(name="pexp", bufs=4))
        otp = ctx.enter_context(tc.tile_pool(name="otp", bufs=8))
        misc = ctx.enter_context(tc.tile_pool(name="misc", bufs=2))
        stage = ctx.enter_context(tc.tile_pool(name="stage", bufs=1))
        ps_s = ctx.enter_context(tc.tile_pool(name="ps_s", bufs=2,
                                              space="PSUM"))
        ps_acc = ctx.enter_context(tc.tile_pool(name="ps_acc", bufs=2,
                                                space="PSUM"))
        ps_p = ctx.enter_context(tc.tile_pool(name="ps_p", bufs=2,
                                              space="PSUM"))

        # ---- constants (tiles now; DMAs ride inside body 0's queue) ----
        bq_sb = consts.tile([P, NPAIR], F32, tag="bq")
        bk_sb = consts.tile([P, NPAIR], F32, tag="bk")
        bv_sb = consts.tile([P, HD], F32, tag="bv")
        bo_sb = consts.tile([P, NOC], F32, tag="bo")

        # warm the ACT exp table early so the first real exp is cheap
        warm = consts.tile([1, 2], F32, tag="warm")
        nc.gpsimd.memset(warm[0:1, 0:1], 0.0)
        nc.scalar.activation(warm[0:1, 1:2], warm[0:1, 0:1],
                             mybir.ActivationFunctionType.Exp)

        # warm the PE p-state: ~24 dependency-free matmuls keep the PE busy
        # through the initial DMA phase so the first projections run at
        # full clock (TRN2 throttles the PE for ~3us after any idle gap)
        wsrc = consts.tile([P, QB], BF16, tag="wsrc")
        nc.vector.memset(wsrc[:], 0.0)

        # ---- weight tiles (allocated now; DMAs emitted inside body 0 so
        # the x window-0 transfer wins the DMA queue) ----
        wk_sb = wres.tile([P, DC * HD], BF16, tag="wk", name="wk")
        wv_sb = wres.tile([P, DC * HD], BF16, tag="wv", name="wv")
        wq_sb = wres.tile([P, DC * HD], BF16, tag="wq", name="wq")
        wo_sb = wres.tile([P, NPAIR * D_MODEL], BF16, tag="wo", name="wo")

        def emit_wdma(which):
            if which == "kqv":
                for t, d in ((wk_sb, Wk), (wv_sb, Wv), (wq_sb, Wq)):
                    nc.sync.dma_start(
                        t[:].rearrange("p (c n) -> p c n", c=DC),
                        d.ap().rearrange("(c p) n -> p c n", p=P))
                nc.sync.dma_start(bk_sb[:], bk_t.ap())
                nc.sync.dma_start(bv_sb[:], bv_bc.ap())
                nc.sync.dma_start(bq_sb[:], bq_t.ap())
                nc.sync.dma_start(bo_sb[:], bo_t.ap())
            elif which == "warmpe":
                for i in range(24):
                    wps = ps_p.tile([P, QB], F32, tag="pp", name="wps")
                    nc.tensor.matmul(wps[:], wsrc[:, 0:P], wsrc[:],
                                     start=True, stop=True)
            else:
                nc.sync.dma_start(
                    wo_sb[:].rearrange("p (r n) -> p r n", r=NPAIR),
                    Wo.ap().rearrange("(r p) n -> p r n", p=P))

        sbs = dict(bq=bq_sb, bk=bk_sb, bv=bv_sb, bo=bo_sb,
                   wq=wq_sb, wk=wk_sb, wv=wv_sb, wo=wo_sb)
        pools = dict(wres=wres, xres=xres, ktv=ktv, qtp=qtp, pexp=pexp,
                     otp=otp, misc=misc, stage=stage, ps_s=ps_s,
                     ps_acc=ps_acc, ps_p=ps_p)

        pctx = None
        for rep in range(repeat):
            pctx = emit_body(nc, tc, dram, sbs, pools, pctx,
                             is_last=(rep == repeat - 1),
                             emit_wdma=emit_wdma if rep == 0 else None)
        for fn in pctx["carry"]:
            fn()

    nc.compile()
    return nc


def emit_body(nc, tc, dram, sbs, pools, pctx, is_last, emit_wdma=None):
    xres, ktv, qtp = pools["xres"], pools["ktv"], pools["qtp"]
    pexp, otp, misc = pools["pexp"], pools["otp"], pools["misc"]
    stage = pools["stage"]
    ps_s, ps_acc, ps_p = pools["ps_s"], pools["ps_acc"], pools["ps_p"]
    bq_sb, bk_sb, bv_sb, bo_sb = sbs["bq"], sbs["bk"], sbs["bv"], sbs["bo"]
    wq_sb, wk_sb, wv_sb, wo_sb = sbs["wq"], sbs["wk"], sbs["wv"], sbs["wo"]

    xt_dram3 = dram["xT"].ap().rearrange("(c p) s -> p c s", p=P)
    yt_dram3 = dram["YT"].ap().rearrange("(n p) s -> p n s", p=P)

    def w3(t, cols=HD):
        return t[:].rearrange("p (c n) -> p c n", c=DC)

    wq3, wk3, wv3 = w3(wq_sb), w3(wk_sb), w3(wv_sb)
    wo3 = wo_sb[:].rearrange("p (r n) -> p r n", r=NPAIR)
    bv3 = bv_sb[:].rearrange("p (h d) -> p h d", h=HL)

    def alloc_xkv():
        xt = xres.tile([P, DC * SEQ], BF16, tag="xt", name="xt")
        return (xt[:].rearrange("p (c s) -> p c s", c=DC),
                [ktv.tile([P, SEQ], BF16, tag=f"kt{r}", name=f"kt{r}")
                 for r in range(NPAIR)],
                [ktv.tile([P, HL * VBLK], FP8, tag=f"v2_{k}", name=f"v2_{k}")
                 for k in range(KCP)])

    first = pctx is None
    if first:
        # body 0: own x/K/V produced in its own qb0; weight loads ride the
        # DMA queue right after x window 0 (wo last: first needed ~iter 70)
        xt3, kt_tiles, v2_tiles = alloc_xkv()
        nc.sync.dma_start(xt3[:, :, bass.ts(0, QB)],
                          xt_dram3[:, :, bass.ts(0, QB)])
        if emit_wdma is not None:
            emit_wdma("kqv")
            emit_wdma("warmpe")
        for w in range(1, NQB):
            sl = bass.ts(w, QB)
            nc.sync.dma_start(xt3[:, :, sl], xt_dram3[:, :, sl])
        if emit_wdma is not None:
            emit_wdma("o")
    else:
        # steady state: x/K^T/V and Q(0,0) were prefetched by the previous
        # body during its ACT-rich later q-blocks
        xt3, kt_tiles, v2_tiles = pctx["xt3"], pctx["kt"], pctx["v2"]

    # ---- projection group emitters (PE fillers) ----
    # All groups come in 4-matmul halves so no single filler exceeds ~0.9us
    # of PE time — the exp stream's PSUM runway is only ~2 tiles deep.
    kps_hold = {}

    def k_half(pr, w, half, x3=None, kts=None):
        x3 = xt3 if x3 is None else x3
        kts = kt_tiles if kts is None else kts
        sl = bass.ts(w, QB)
        if half == 0:
            kps_hold[(pr, w)] = ps_p.tile([P, QB], F32, tag="pp", name="kps")
        kps = kps_hold[(pr, w)]
        for c in range(4 * half, 4 * half + 4):
            nc.tensor.matmul(kps[:], wk3[:, c, bass.ts(pr, P)],
                             x3[:, c, sl],
                             start=(c == 0), stop=(c == DC - 1))
        if half == 1:
            nc.vector.tensor_scalar_add(kts[pr][:, sl], kps[:],
                                        bk_sb[:, pr:pr + 1])
            del kps_hold[(pr, w)]

    def k_group(pr, w, x3=None, kts=None):
        k_half(pr, w, 0, x3, kts)
        k_half(pr, w, 1, x3, kts)

    qt_map = {}
    if not first:
        qt_map[(0, 0)] = pctx["qt00"]
    qps_hold = {}

    def q_half(qb, pr, half, x3=None, store=None):
        x3 = xt3 if x3 is None else x3
        if half == 0:
            qps_hold[(qb, pr)] = ps_p.tile([P, QB], F32, tag="pp",
                                           name="qps")
        qps = qps_hold[(qb, pr)]
        for c in range(4 * half, 4 * half + 4):
            nc.tensor.matmul(qps[:], wq3[:, c, bass.ts(pr, P)],
                             x3[:, c, bass.ts(qb, QB)],
                             start=(c == 0), stop=(c == DC - 1))
        if half == 1:
            qt = qtp.tile([P, QB], BF16, tag="qt", name="qt")
            nc.vector.tensor_scalar_add(qt[:], qps[:], bq_sb[:, pr:pr + 1])
            del qps_hold[(qb, pr)]
            if store is None:
                qt_map[(qb, pr)] = qt
            else:
                store[0] = qt

    def q_group(qb, pr, x3=None, store=None):
        q_half(qb, pr, 0, x3, store)
        q_half(qb, pr, 1, x3, store)

    # V projection in quarter-groups (4 matmuls each) so the PE load can be
    # spread finely between score/PV iterations
    vps_hold = {}

    def v_quarter(kcp, t, half, x3=None, v2s=None):
        x3 = xt3 if x3 is None else x3
        v2s = v2_tiles if v2s is None else v2s
        k = 2 * kcp + t
        if half == 0:
            vps_hold[(kcp, t)] = ps_p.tile([P, HD], F32, tag="pp", name="vps")
        vps = vps_hold[(kcp, t)]
        for c in range(4 * half, 4 * half + 4):
            nc.tensor.matmul(vps[:], x3[:, c, bass.ts(k, P)],
                             wv3[:, c, :],
                             start=(c == 0), stop=(c == DC - 1))
        if half == 1:
            v2t = v2s[kcp]
            v2h = v2t[:].rearrange("p (h z) -> p h z", h=HL)
            nc.vector.tensor_add(
                v2h[:, :, t * 80: t * 80 + DK],
                vps[:].rearrange("p (h d) -> p h d", h=HL), bv3)
            del vps_hold[(kcp, t)]
            if t == 1:
                # ones columns (softmax denominator rides the PV matmul)
                v2g = v2t[:].rearrange("p (g z) -> p g z", g=2 * HL)
                nc.vector.memset(v2g[:, :, DK:DK + 1], 1.0)

    ot_map = {}
    yhalf = {}

    def o_group(qb, n):
        yps = ps_p.tile([P, QB], F32, tag="pp", name="yps")
        for pr in range(NPAIR):
            nc.tensor.matmul(yps[:], wo3[:, pr, bass.ts(n, P)],
                             ot_map[(qb, pr)][:],
                             start=(pr == 0), stop=(pr == NPAIR - 1))
        ysb = misc.tile([P, QB], F32, tag=f"ysb{n % 4}", name="ysb")
        nc.vector.tensor_scalar_add(ysb[:], yps[:], bo_sb[:, n:n + 1])
        nc.sync.dma_start(yt_dram3[:, n, bass.ts(qb, QB)], ysb[:])

    # last q-block's out-projection is split so pairs {0,1} are folded in
    # while pairs {2,3} still run; only the second half lands in the tail
    def o_half1(qb, n):
        yps = ps_p.tile([P, QB], F32, tag="pp", name="yps")
        for pr in (0, 1):
            nc.tensor.matmul(yps[:], wo3[:, pr, bass.ts(n, P)],
                             ot_map[(qb, pr)][:],
                             start=(pr == 0), stop=(pr == 1))
        ya = stage.tile([P, QB], F32, tag=f"ya{n}", name="ya")
        nc.vector.tensor_scalar_add(ya[:], yps[:], bo_sb[:, n:n + 1])
        yhalf[(qb, n)] = ya

    def o_half2(qb, n):
        yps = ps_p.tile([P, QB], F32, tag="pp", name="yps")
        for pr in (2, 3):
            nc.tensor.matmul(yps[:], wo3[:, pr, bass.ts(n, P)],
                             ot_map[(qb, pr)][:],
                             start=(pr == 2), stop=(pr == 3))
        ysb = misc.tile([P, QB], F32, tag=f"ysb{n % 4}", name="ysb")
        nc.vector.tensor_add(ysb[:], yps[:], yhalf[(qb, n)][:])
        nc.sync.dma_start(yt_dram3[:, n, bass.ts(qb, QB)], ysb[:])

    # ---- filler schedule: (due_global_iter, seq, fn) ----
    fillers = []

    def add(due, fn):
        fillers.append((due, len(fillers), fn))

    # previous body's trailing out-projection halves land first (their ot
    # tiles are ready; the PE has slack while ACT drains its exp backlog)
    if not first:
        for fn in pctx["carry"]:
            add(0, fn)

    if first:
        # body 0 produces its own K/V/Q(0,0) just-in-time during qb0
        for w in range(1, NQB):
            for half in (0, 1):
                add(3 * w - 1 + half, lambda pr=0, w=w, half=half:
                    k_half(pr, w, half))
        for pr in range(1, NPAIR):
            for w in range(NQB):
                for half in (0, 1):
                    add(pr * ITERS_PER_PAIR + 3 * w - 5 + half,
                        lambda pr=pr, w=w, half=half: k_half(pr, w, half))
        for kcp in range(KCP):
            for t in (0, 1):
                for half in (0, 1):
                    idx = 4 * kcp + 2 * t + half
                    add(max(0, (idx - 2) // 2),
                        lambda kcp=kcp, t=t, half=half: v_quarter(kcp, t, half))
    # Q: due a few iters before pair (qb, pr) starts
    for qb in range(NQB):
        for pr in range(NPAIR):
            if (qb, pr) == (0, 0):
                continue
            base = (qb * NPAIR + pr) * ITERS_PER_PAIR - 7
            add(base, lambda qb=qb, pr=pr: q_half(qb, pr, 0))
            add(base + 1, lambda qb=qb, pr=pr: q_half(qb, pr, 1))
    # out-projection for qb spread across qb+1 (odd offsets so they don't
    # land on the same iteration as the K/V prefetch fillers); last qb:
    # first half during its own pair 2/3, second half carried into the
    # next body
    for qb in range(NQB - 1):
        for n in range(NOC):
            add((qb + 1) * NPAIR * ITERS_PER_PAIR + 8 * n + 5,
                lambda qb=qb, n=n: o_group(qb, n))
    last_pair2 = ((NQB - 1) * NPAIR + 2) * ITERS_PER_PAIR
    for n in range(NOC):
        add(last_pair2 + 3 * n + 2, lambda n=n: o_half1(NQB - 1, n))

    # ---- prefetch the NEXT body's x/K^T/V/Q(0,0) during qb1-qb3 ----
    pctx_next = {"carry": []}
    if not is_last:
        nxt3, nkt, nv2 = alloc_xkv()
        pctx_next.update(xt3=nxt3, kt=nkt, v2=nv2)

        def next_x_dma(w, x3=nxt3):
            sl = bass.ts(w, QB)
            nc.sync.dma_start(x3[:, :, sl], xt_dram3[:, :, sl])

        for w in range(NQB):
            add(64 + 2 * w, lambda w=w: next_x_dma(w))
        for pr in range(NPAIR):
            for w in range(NQB):
                for half in (0, 1):
                    add(128 + 4 * (4 * pr + w) + 2 * half,
                        lambda pr=pr, w=w, half=half:
                            k_half(pr, w, half, x3=nxt3, kts=nkt))
        for kcp in range(KCP):
            for t in (0, 1):
                for half in (0, 1):
                    idx = 4 * kcp + 2 * t + half
                    add(192 + 2 * idx,
                        lambda kcp=kcp, t=t, half=half:
                            v_quarter(kcp, t, half, x3=nxt3, v2s=nv2))
        qt00_box = [None]
        add(248, lambda: q_half(0, 0, 0, x3=nxt3, store=qt00_box))
        add(249, lambda: q_half(0, 0, 1, x3=nxt3, store=qt00_box))
        pctx_next["qt00_box"] = qt00_box

    fillers.sort()
    fidx = [0]

    def emit_due(it):
        while fidx[0] < len(fillers) and fillers[fidx[0]][0] <= it:
            fillers[fidx[0]][2]()
            fidx[0] += 1

    # ---- normalize: O^T rows 0:64 scaled by 1/denominator (row 64) ----
    def normalize(qb, pr, hh, acc):
        if hh == 0:
            ot_map[(qb, pr)] = otp.tile([P, QB], BF16, tag="ot", name="ot")
        ra = misc.tile([1, QB], F32, tag="ra", name="ra")
        nc.vector.reciprocal(ra[:], acc[DK:DK + 1, :])
        bc = misc.tile([DK, QB], F32, tag="bc", name="bc")
        nc.gpsimd.partition_broadcast(bc[:], ra[:], channels=DK)
        nc.vector.tensor_mul(ot_map[(qb, pr)][hh * DK:(hh + 1) * DK, :],
                             acc[0:DK, :], bc[:])

    # ---- main attention stream ----
    state = {"pending": None}
    acc_by_hh = {}

    def flush_pending():
        p = state["pending"]
        if p is None:
            return
        ef, qb, pr, hh, kcp = p
        if kcp == 0:
            acc_by_hh[hh] = ps_acc.tile([DK + 1, QB], F32, tag="acc",
                                        name="acc")
        acc = acc_by_hh[hh]
        h = 2 * pr + hh
        v2t = v2_tiles[kcp]
        lhsT = (v2t[:, h * VBLK:(h + 1) * VBLK]
                .rearrange("p (t c) -> p t c", t=2)[:, :, 0:DK + 1])
        rhs = ef[:].rearrange("p (t n) -> p t n", t=2)
        nc.tensor.matmul(acc[:], lhsT, rhs, start=(kcp == 0),
                         stop=(kcp == KCP - 1),
                         perf_mode=mybir.MatmulPerfMode.DoubleRow)
        if kcp == KCP - 1:
            normalize(qb, pr, hh, acc)
        state["pending"] = None

    # prologue: body 0 needs its first K window + first Q before any scores
    # (steady-state bodies got them via prefetch)
    if first:
        k_group(0, 0)
        q_group(0, 0)

    git = 0
    for qb in range(NQB):
        for pr in range(NPAIR):
            for it_p in range(ITERS_PER_PAIR):
                kcp, hh = it_p // 2, it_p % 2
                qt = qt_map[(qb, pr)]
                swide = ps_s.tile([P, 2 * QB], F32, tag="sw", name="sw")
                hsl = slice(hh * DK, (hh + 1) * DK)
                for t in (0, 1):
                    k = 2 * kcp + t
                    nc.tensor.matmul(swide[:, t * QB:(t + 1) * QB],
                                     kt_tiles[pr][hsl, bass.ts(k, P)],
                                     qt[hsl, :], start=True, stop=True)
                ef = pexp.tile([P, 2 * QB], FP8, tag="ef", name="ef")
                nc.scalar.activation(ef[:], swide[:],
                                     mybir.ActivationFunctionType.Exp,
                                     scale=float(EXP_SCALE))
                flush_pending()
                state["pending"] = (ef, qb, pr, hh, kcp)
                emit_due(git)
                git += 1

    # epilogue: last PV + normalize, remaining fillers; the trailing
    # out-projection halves carry into the next body's filler stream
    flush_pending()
    emit_due(1 << 30)
    if not is_last:
        pctx_next["qt00"] = pctx_next.pop("qt00_box")[0]
    pctx_next["carry"] = [
        (lambda n=n: o_half2(NQB - 1, n)) for n in range(NOC)]
    return pctx_next


_CACHE = {}


def _get_nc():
    if "nc" not in _CACHE:
        _CACHE["nc"] = build_bass()
    return _CACHE["nc"]


def host_prep(x, Wq, bq, Wk, bk, Wv, bv, Wo, bo):
    """Build the 8 per-core input maps."""
    bf = ml_dtypes.bfloat16
    in_maps = []
    for core in range(N_CORES):
        b, g = divmod(core, 2)
        lo, hi = g * HD, (g + 1) * HD
        in_maps.append({
            "xT": np.ascontiguousarray(x[b].T).astype(bf),
            "Wq": np.ascontiguousarray(Wq[:, lo:hi]).astype(bf),
            "Wk": np.ascontiguousarray(Wk[:, lo:hi]).astype(bf),
            "Wv": np.ascontiguousarray(Wv[:, lo:hi]).astype(bf),
            "Wo": np.ascontiguousarray(Wo[lo:hi, :]).astype(bf),
            "bq_t": np.ascontiguousarray(bq[lo:hi].reshape(NPAIR, P).T),
            "bk_t": np.ascontiguousarray(bk[lo:hi].reshape(NPAIR, P).T),
            "bv_bc": np.broadcast_to(bv[lo:hi], (P, HD)).copy(),
            "bo_t": np.ascontiguousarray((bo * 0.5).reshape(NOC, P).T),
        })
    return in_maps


def host_gather(results):
    """Sum head-group partials and transpose back to [B, S, D]."""
    out = np.empty((BATCH, SEQ, D_MODEL), dtype=np.float32)
    for b in range(BATCH):
        yt = results[2 * b]["YT"] + results[2 * b + 1]["YT"]
        out[b] = yt.T
    return out


def kernel(x, Wq, bq, Wk, bk, Wv, bv, Wo, bo):
    nc = _get_nc()
    in_maps = host_prep(x, Wq, bq, Wk, bk, Wv, bv, Wo, bo)
    res = run_bass_kernel_spmd(nc, in_maps, core_ids=list(range(N_CORES)))
    return host_gather(res.results)
